# revision 1
# baseline (speedup 1.0000x reference)
"""Trainium2 Bass kernel for nn_ChannelSelfAttention.

Reference computation (per batch sample b):
    xt   = x[b].T                          # [C, L]
    q    = xt @ Wq.T + bq                  # [C, H]
    kv   = xt @ Wkv.T + bkv                # [C, 2H] -> k, v
    attn = (q * H**-0.5) @ k.T             # [C, C]  (no softmax)
    y    = attn @ v                        # [C, H]
    g    = mean(y, axis=-1)                # [C]
    out[b] = x[b] * g[None, :]             # [L, C]

Sharding: data-parallel over B across 8 cores (4 samples per core);
weights replicated.

HBM-bound problem with a 2e-2 rel-err gate, so all HBM I/O is bf16
(host casts inputs, kernel stores bf16, host upcasts the output):
17.5 MiB per core vs 35 MiB in f32 -> ~46 us at the ~420 GB/s
single-ring DMA rate.

Device-side structure:
  - Tile-framework dependencies are TILE-granular, so x / W^T / out are
    split into per-l-chunk tiles (512 KiB / 384 KiB / 512 KiB DMAs,
    4 KiB descriptors): each qkv chunk-matmul gates only on its own
    chunk's DMA, and each output chunk stores as soon as its gate
    multiply finishes.
  - All loads ride the sync HWDGE ring, interleaved in consumption
    order (wT0, x0c0, wT1, x0c1, ... then x1..x3); constants ride the
    scalar ring (a few KiB).  Stores also ride the sync ring, emitted
    after all loads: ring FIFO then costs nothing (total bytes / rate
    is unchanged) and, critically, store issues do NOT occupy the ACT
    engine, whose in-order stream would otherwise stall the PSUM->SBUF
    copies of later samples behind store-gate semaphore waits.
  - qkv is computed x-stationary: lhsT = x chunk [128 l, 128 c-group],
    rhs = W_all^T chunk [128 l, 192]; 2 groups x 32 chunks x 192
    streamed columns = 12288 PE cycles/sample (the MAC optimum), and
    q, k, v land in natural [c, h] layout.  The bias (Wq and bq
    pre-scaled by H^-0.5 on the host) enters as a K=1 outer-product
    matmul that opens each PSUM accumulation group.
  - mean-over-H commutes into v: g[c] = sum_d attn^T[d,c] * vbar[d],
    vbar = mean_h v, so y is never materialized.  vbar comes from a
    DVE free-axis reduce of natural v; 1/H is folded into the ones
    lhsT of the final broadcast matmul, which lands g on all 128
    partitions.
  - q^T/k^T via 4 PE transposes; attn^T = k^T-chunk x q^T; at_sb =
    attn^T * vbar via DVE tensor_scalar (PSUM -> bf16 SBUF).
  - gate: per-chunk DVE tensor_tensor, all-bf16 (packed 2x mode), with
    g broadcast along j via a stride-0 AP.
  - PE warm-up: ~32 junk matmuls on a memset scratch right after the
    preamble keep the HAM activity monitor busy so the PE clock is at
    2.4 GHz (not the cold 1.2 GHz) when the first real matmul issues.
"""

import numpy as np
import ml_dtypes

import concourse.bass as bass
import concourse.mybir as mybir
import concourse.tile as tile
from concourse import bacc
from concourse.bass_utils import run_bass_kernel_spmd

B, L, C, H = 32, 4096, 256, 64
N_CORES = 8
B_LOC = B // N_CORES          # samples per core
P = 128                       # SBUF partitions
JC = 8                        # L-rows per partition per chunk (4KB bf16 descs)
NCH = L // (P * JC)           # l-chunks per sample (4)
GC = C // P                   # c-groups (2)
TH = 3 * H                    # 192 = q|k|v
BF16 = mybir.dt.bfloat16
F32 = mybir.dt.float32
SCALE = float(H) ** -0.5
BF = ml_dtypes.bfloat16
N_WARM = 18                   # PE warm-up junk matmuls


def _emit(tc: "tile.TileContext", x_d, wT_d, bias_d, id_d, ones1_d,
          onesg_d, out_d) -> None:
    nc = tc.nc
    with (
        tc.tile_pool(name="singles", bufs=1) as singles,
        tc.tile_pool(name="xin", bufs=B_LOC) as xin,
        tc.tile_pool(name="xout", bufs=3) as xout,
        tc.tile_pool(name="small", bufs=2) as small,
        tc.tile_pool(name="psA", bufs=2, space="PSUM") as psA,
        tc.tile_pool(name="psA2", bufs=2, space="PSUM") as psA2,
        tc.tile_pool(name="psB", bufs=2, space="PSUM") as psB,
        tc.tile_pool(name="psC", bufs=2, space="PSUM") as psC,
    ):
        # ---- constants on the scalar ring (a few KiB, land early) ----
        bias_sb = singles.tile([1, TH], BF16)                # (bq*scale)|bkv
        nc.scalar.dma_start(out=bias_sb, in_=bias_d[:])
        ones1 = singles.tile([1, P], BF16)                   # ones row
        nc.scalar.dma_start(out=ones1, in_=ones1_d[:])
        ident = singles.tile([P, P], BF16)
        nc.scalar.dma_start(out=ident, in_=id_d[:])
        onesg = singles.tile([P, P], BF16)                   # filled with 1/H
        nc.scalar.dma_start(out=onesg, in_=onesg_d[:])

        # ---- PE warm-up: junk matmuls on zeroed scratch so the HAM
        # clock-gate is at 2.4 GHz when real work arrives ----
        scratch = singles.tile([P, P + C], BF16)
        nc.vector.memset(scratch, 0.0)
        psj = psA.tile([P, TH], F32, tag="qkv0", name="psj")
        for _ in range(N_WARM):
            nc.tensor.matmul(psj, lhsT=scratch[:, 0:P],
                             rhs=scratch[:, P : P + TH])

        # ---- loads on the sync ring, in consumption order ----
        wT_src = wT_d[:].rearrange("(n p j) h -> n p (j h)", p=P, j=JC)
        x_srcs = [x_d[b].rearrange("(n p j) c -> n p (j c)", p=P, j=JC)
                  for b in range(B_LOC)]
        out_dsts = [out_d[b].rearrange("(n p j) c -> n p (j c)", p=P, j=JC)
                    for b in range(B_LOC)]
        wt0 = singles.tile([P, 1, JC * TH], BF16)            # 384 KiB
        nc.sync.dma_start(out=wt0, in_=wT_src[0:1].rearrange("n p x -> p n x"))
        wtR = singles.tile([P, NCH - 1, JC * TH], BF16)      # 1.1 MiB
        nc.sync.dma_start(
            out=wtR, in_=wT_src[1:NCH].rearrange("n p x -> p n x")
        )
        wts = [wt0[:, 0]] + [wtR[:, n - 1] for n in range(1, NCH)]
        xs = [[None] * 2 for _ in range(B_LOC)]
        for b in range(B_LOC):
            for h in range(2):
                xs[b][h] = xin.tile([P, NCH // 2, JC * C], BF16,
                                    tag=f"x{h}", name=f"x_b{b}_h{h}")
                nc.sync.dma_start(
                    out=xs[b][h],
                    in_=x_srcs[b][h * (NCH // 2) : (h + 1) * (NCH // 2)]
                    .rearrange("n p x -> p n x"),
                )

        def qkv_stage(b):
            pq = [psA.tile([P, TH], F32, tag="qkv0", name="pq0"),
                  psA2.tile([P, TH], F32, tag="qkv1", name="pq1")]
            for g in range(GC):
                nc.tensor.matmul(
                    pq[g], lhsT=ones1, rhs=bias_sb, start=True, stop=False,
                )
            for n in range(NCH):
                for j in range(JC):
                    for g in range(GC):
                        nc.tensor.matmul(
                            pq[g],
                            lhsT=xs[b][n // 2][
                                :, n % 2,
                                j * C + g * P : j * C + (g + 1) * P],
                            rhs=wts[n][:, j * TH : (j + 1) * TH],
                            start=False,
                            stop=(n == NCH - 1 and j == JC - 1),
                        )
            qkv_sb = small.tile([P, GC, TH], BF16, tag="qkv_sb")
            for g in range(GC):
                nc.scalar.copy(qkv_sb[:, g], pq[g])
            return qkv_sb

        def tail_stage(b, qkv_sb):
            # vbar[d] = sum_h v[d, h]  (1/H folded into onesg)
            vbar_sb = small.tile([P, GC, 1], F32, tag="vbar")
            for g in range(GC):
                nc.vector.tensor_reduce(
                    out=vbar_sb[:, g], in_=qkv_sb[:, g, 2 * H : TH],
                    axis=mybir.AxisListType.X, op=mybir.AluOpType.add,
                )
            # q^T, k^T [64, 256] via PE transpose
            psum_t = psB.tile([H, 2, C], BF16, tag="qkt")
            for g in range(GC):
                nc.tensor.transpose(
                    psum_t[:, 0, g * P : (g + 1) * P],
                    qkv_sb[:, g, 0:H], ident,
                )
                nc.tensor.transpose(
                    psum_t[:, 1, g * P : (g + 1) * P],
                    qkv_sb[:, g, H : 2 * H], ident,
                )
            qkt_sb = small.tile([H, 2, C], BF16, tag="qkt_sb")
            nc.scalar.copy(qkt_sb, psum_t)
            qT = qkt_sb[:, 0]
            kT = qkt_sb[:, 1]

            # attn^T[d, c] = sum_h k^T[h, d] q^T[h, c]
            psum_at = psC.tile([P, GC, C], F32, tag="at")
            for d in range(GC):
                nc.tensor.matmul(
                    psum_at[:, d], lhsT=kT[:, d * P : (d + 1) * P], rhs=qT,
                )
            # at_sb = attn^T * vbar (per-partition scalar), PSUM -> bf16
            at_sb = small.tile([P, GC, C], BF16, tag="at_sb")
            for d in range(GC):
                nc.scalar.activation(
                    out=at_sb[:, d], in_=psum_at[:, d],
                    func=mybir.ActivationFunctionType.Copy,
                    scale=vbar_sb[:, d],
                )
            # g[c] = (1/H) sum_d at_sb[d, c], broadcast to 128 partitions.
            # Reuses the at PSUM region (its values were already drained to
            # at_sb by the activations; Tile orders the write-after-read).
            psum_g = psum_at[:, 0]
            for d in range(GC):
                nc.tensor.matmul(
                    psum_g, lhsT=onesg, rhs=at_sb[:, d],
                    start=(d == 0), stop=(d == GC - 1),
                )
            g_sb = small.tile([P, C], BF16, tag="g_sb")
            nc.scalar.copy(g_sb, psum_g)

            # gate + store per chunk (each store flows as soon as its
            # chunk's gate multiply is done)
            g_bc = bass.AP(
                tensor=g_sb.tensor,
                offset=g_sb.offset,
                ap=[list(g_sb.ap[0]), [0, NCH // 2], [0, JC], list(g_sb.ap[1])],
            )
            for h in range(2):
                o_t = xout.tile([P, NCH // 2, JC * C], BF16,
                                tag=f"o{h}", name=f"o_b{b}_h{h}")
                nc.vector.tensor_tensor(
                    out=o_t.rearrange("p n (j c) -> p n j c", j=JC),
                    in0=xs[b][h].rearrange("p n (j c) -> p n j c", j=JC),
                    in1=g_bc,
                    op=mybir.AluOpType.mult,
                )
                nc.sync.dma_start(
                    out=out_dsts[b][h * (NCH // 2) : (h + 1) * (NCH // 2)]
                    .rearrange("n p x -> p n x"),
                    in_=o_t,
                )

        for b in range(B_LOC):
            tail_stage(b, qkv_stage(b))


def build():
    nc = bacc.Bacc(
        "TRN2", target_bir_lowering=False, debug=False, num_devices=N_CORES
    )
    x_d = nc.dram_tensor("x", [B_LOC, L, C], BF16, kind="ExternalInput")
    wT_d = nc.dram_tensor("wT", [L, TH], BF16, kind="ExternalInput")
    bias_d = nc.dram_tensor("bias", [1, TH], BF16, kind="ExternalInput")
    id_d = nc.dram_tensor("ident", [P, P], BF16, kind="ExternalInput")
    ones1_d = nc.dram_tensor("ones1", [1, P], BF16, kind="ExternalInput")
    onesg_d = nc.dram_tensor("onesg", [P, P], BF16, kind="ExternalInput")
    out_d = nc.dram_tensor("out", [B_LOC, L, C], BF16, kind="ExternalOutput")
    with tile.TileContext(nc) as tc:
        _emit(tc, x_d, wT_d, bias_d, id_d, ones1_d, onesg_d, out_d)
    nc.compile()
    return nc


_nc_cache = None


def _get_nc():
    global _nc_cache
    if _nc_cache is None:
        _nc_cache = build()
    return _nc_cache


def make_in_maps(x, Wq, bq, Wkv, bkv):
    x_bf = np.asarray(x, dtype=np.float32).astype(BF)
    wT = np.ascontiguousarray(
        np.concatenate(
            [np.asarray(Wq, np.float32) * SCALE, np.asarray(Wkv, np.float32)],
            axis=0,
        ).T.astype(BF)
    )
    bias = np.concatenate(
        [np.asarray(bq, np.float32) * SCALE, np.asarray(bkv, np.float32)]
    )[None].astype(BF)
    ident = np.eye(P, dtype=BF)
    ones1 = np.ones((1, P), dtype=BF)
    onesg = np.full((P, P), 1.0 / H, dtype=BF)
    return [
        {
            "x": np.ascontiguousarray(x_bf[i * B_LOC : (i + 1) * B_LOC]),
            "wT": wT,
            "bias": bias,
            "ident": ident,
            "ones1": ones1,
            "onesg": onesg,
        }
        for i in range(N_CORES)
    ]


def run(inputs, **spmd_kwargs):
    """Run on hardware; returns (full_output, BassKernelResults)."""
    nc = _get_nc()
    in_maps = make_in_maps(**inputs)
    res = run_bass_kernel_spmd(nc, in_maps, list(range(N_CORES)), **spmd_kwargs)
    out = np.concatenate([r["out"] for r in res.results], axis=0)
    return np.asarray(out).astype(np.float32), res


def kernel(**inputs) -> np.ndarray:
    out, _ = run(inputs)
    return out



# revision 2
# speedup vs baseline: 1.0129x; 1.0129x over previous
"""Trainium2 Bass kernel for nn_ChannelSelfAttention.

Reference computation (per batch sample b):
    xt   = x[b].T                          # [C, L]
    q    = xt @ Wq.T + bq                  # [C, H]
    kv   = xt @ Wkv.T + bkv                # [C, 2H] -> k, v
    attn = (q * H**-0.5) @ k.T             # [C, C]  (no softmax)
    y    = attn @ v                        # [C, H]
    g    = mean(y, axis=-1)                # [C]
    out[b] = x[b] * g[None, :]             # [L, C]

Sharding: data-parallel over B across 8 cores (4 samples per core);
weights replicated.

HBM-bound problem with a 2e-2 rel-err gate, so all HBM I/O is bf16
(host casts inputs, kernel stores bf16, host upcasts the output):
17.5 MiB per core, and the DMA stream runs at the per-NC HBM wall
(~360-380 GB/s effective) -> ~48 us of unavoidable ring time.

Schedule (v2): the ring must never starve and the tail must be short.
  - Constants (bias/ones/ident/onesg) ride the GPSIMD (SWDGE) queue so
    the sync HWDGE ring carries only bulk x/wT/out traffic and the ACT
    engine never issues DMAs.
  - Loads on sync in consumption order (wT then x per sample); all load
    issues precede all store issues so the SP sequencer's store-gate
    semaphore waits can't delay any load.
  - PE emission is software-pipelined: qkv(b+1) half-0 matmuls are
    emitted between qkv(b)'s PSUM drain and sample b's tail matmuls
    (transposes/attn/g), and half-1 after, so PE never idles waiting
    for the ACT copy chain of sample b.
  - The gate multiply is chunked 4x512KB per sample, each chunk's store
    issued immediately: stores enqueue early (ring never starves at the
    end) and the final store is a small chunk right behind the last DVE
    multiply instead of a 1 MiB blob.
  - mean-over-H commutes into v: g[c] = sum_d attn^T[d,c] * vbar[d],
    vbar = mean_h v, so y is never materialized; 1/H is folded into the
    ones lhsT of the broadcast matmul that lands g on all partitions.
  - PE warm-up: ~28 junk matmuls on a memset scratch bridge the gap to
    the first real matmul so the PE clock is at full speed by then.
"""

import numpy as np
import ml_dtypes

import concourse.bass as bass
import concourse.mybir as mybir
import concourse.tile as tile
from concourse import bacc
from concourse.bass_utils import run_bass_kernel_spmd

B, L, C, H = 32, 4096, 256, 64
N_CORES = 8
B_LOC = B // N_CORES          # samples per core
P = 128                       # SBUF partitions
JC = 8                        # L-rows per partition per chunk (4KB bf16 descs)
NCH = L // (P * JC)           # l-chunks per sample (4)
GC = C // P                   # c-groups (2)
TH = 3 * H                    # 192 = q|k|v
BF16 = mybir.dt.bfloat16
F32 = mybir.dt.float32
SCALE = float(H) ** -0.5
BF = ml_dtypes.bfloat16
N_WARM = 28                   # PE warm-up junk matmuls


def _emit(tc: "tile.TileContext", x_d, wT_d, bias_d, id_d, ones1_d,
          onesg_d, out_d) -> None:
    nc = tc.nc
    with (
        tc.tile_pool(name="singles", bufs=1) as singles,
        tc.tile_pool(name="xin", bufs=B_LOC) as xin,
        tc.tile_pool(name="xout", bufs=10) as xout,
        tc.tile_pool(name="small", bufs=2) as small,
        tc.tile_pool(name="psA", bufs=2, space="PSUM") as psA,
        tc.tile_pool(name="psA2", bufs=2, space="PSUM") as psA2,
        tc.tile_pool(name="psB", bufs=2, space="PSUM") as psB,
        tc.tile_pool(name="psC", bufs=2, space="PSUM") as psC,
    ):
        # ---- constants on the GPSIMD (SWDGE) queue: tiny, lands early,
        # steals nothing from the sync ring or the ACT/SP sequencers ----
        bias_sb = singles.tile([1, TH], BF16)                # (bq*scale)|bkv
        nc.gpsimd.dma_start(out=bias_sb, in_=bias_d[:])
        ones1 = singles.tile([1, P], BF16)                   # ones row
        nc.gpsimd.dma_start(out=ones1, in_=ones1_d[:])
        ident = singles.tile([P, P], BF16)
        nc.gpsimd.dma_start(out=ident, in_=id_d[:])
        onesg = singles.tile([P, P], BF16)                   # filled with 1/H
        nc.gpsimd.dma_start(out=onesg, in_=onesg_d[:])

        # ---- PE warm-up: junk matmuls on zeroed scratch so the HAM
        # clock-gate is at full speed when real work arrives ----
        scratch = singles.tile([P, P + C], BF16)
        nc.vector.memset(scratch, 0.0)
        psj = psA.tile([P, TH], F32, tag="qkv0", name="psj")
        for _ in range(N_WARM):
            nc.tensor.matmul(psj, lhsT=scratch[:, 0:P],
                             rhs=scratch[:, P : P + TH])

        # ---- bulk loads on the sync ring, in consumption order ----
        wT_src = wT_d[:].rearrange("(n p j) h -> n p (j h)", p=P, j=JC)
        x_srcs = [x_d[b].rearrange("(n p j) c -> n p (j c)", p=P, j=JC)
                  for b in range(B_LOC)]
        out_dsts = [out_d[b].rearrange("(n p j) c -> n p (j c)", p=P, j=JC)
                    for b in range(B_LOC)]
        wt0 = singles.tile([P, 1, JC * TH], BF16)            # 384 KiB
        nc.sync.dma_start(out=wt0, in_=wT_src[0:1].rearrange("n p x -> p n x"))
        wtR = singles.tile([P, NCH - 1, JC * TH], BF16)      # 1.1 MiB
        nc.sync.dma_start(
            out=wtR, in_=wT_src[1:NCH].rearrange("n p x -> p n x")
        )
        wts = [wt0[:, 0]] + [wtR[:, n - 1] for n in range(1, NCH)]
        xs = [[None] * 2 for _ in range(B_LOC)]
        for b in range(B_LOC):
            for h in range(2):
                xs[b][h] = xin.tile([P, NCH // 2, JC * C], BF16,
                                    tag=f"x{h}", name=f"x_b{b}_h{h}")
                nc.sync.dma_start(
                    out=xs[b][h],
                    in_=x_srcs[b][h * (NCH // 2) : (h + 1) * (NCH // 2)]
                    .rearrange("n p x -> p n x"),
                )

        def qkv_bias(b):
            """Open sample b's PSUM accumulation groups with the bias."""
            pq = [psA.tile([P, TH], F32, tag="qkv0", name=f"pq0_b{b}"),
                  psA2.tile([P, TH], F32, tag="qkv1", name=f"pq1_b{b}")]
            for g in range(GC):
                nc.tensor.matmul(
                    pq[g], lhsT=ones1, rhs=bias_sb, start=True, stop=False,
                )
            return pq

        def qkv_half(b, h, pq):
            """Chunk matmuls for l-chunks 2h, 2h+1 of sample b."""
            for n in range(2 * h, 2 * h + 2):
                for j in range(JC):
                    for g in range(GC):
                        nc.tensor.matmul(
                            pq[g],
                            lhsT=xs[b][n // 2][
                                :, n % 2,
                                j * C + g * P : j * C + (g + 1) * P],
                            rhs=wts[n][:, j * TH : (j + 1) * TH],
                            start=False,
                            stop=(n == NCH - 1 and j == JC - 1),
                        )

        def qkv_copy(b, pq):
            qkv_sb = small.tile([P, GC, TH], BF16, tag="qkv_sb")
            for g in range(GC):
                nc.scalar.copy(qkv_sb[:, g], pq[g])
            return qkv_sb

        def tail_stage(b, qkv_sb):
            """vbar, transposes, attn^T, g broadcast for sample b."""
            # vbar[d] = sum_h v[d, h]  (1/H folded into onesg)
            vbar_sb = small.tile([P, GC, 1], F32, tag="vbar")
            for g in range(GC):
                nc.vector.tensor_reduce(
                    out=vbar_sb[:, g], in_=qkv_sb[:, g, 2 * H : TH],
                    axis=mybir.AxisListType.X, op=mybir.AluOpType.add,
                )
            # q^T, k^T [64, 256] via PE transpose
            psum_t = psB.tile([H, 2, C], BF16, tag="qkt")
            for g in range(GC):
                nc.tensor.transpose(
                    psum_t[:, 0, g * P : (g + 1) * P],
                    qkv_sb[:, g, 0:H], ident,
                )
                nc.tensor.transpose(
                    psum_t[:, 1, g * P : (g + 1) * P],
                    qkv_sb[:, g, H : 2 * H], ident,
                )
            qkt_sb = small.tile([H, 2, C], BF16, tag="qkt_sb")
            nc.scalar.copy(qkt_sb, psum_t)
            qT = qkt_sb[:, 0]
            kT = qkt_sb[:, 1]

            # attn^T[d, c] = sum_h k^T[h, d] q^T[h, c]
            psum_at = psC.tile([P, GC, C], F32, tag="at")
            for d in range(GC):
                nc.tensor.matmul(
                    psum_at[:, d], lhsT=kT[:, d * P : (d + 1) * P], rhs=qT,
                )
            # at_sb = attn^T * vbar (per-partition scalar), PSUM -> bf16
            at_sb = small.tile([P, GC, C], BF16, tag="at_sb")
            for d in range(GC):
                nc.scalar.activation(
                    out=at_sb[:, d], in_=psum_at[:, d],
                    func=mybir.ActivationFunctionType.Copy,
                    scale=vbar_sb[:, d],
                )
            # g[c] = (1/H) sum_d at_sb[d, c], broadcast to 128 partitions.
            # Reuses the at PSUM region (drained to at_sb by the
            # activations; Tile orders the write-after-read).
            psum_g = psum_at[:, 0]
            for d in range(GC):
                nc.tensor.matmul(
                    psum_g, lhsT=onesg, rhs=at_sb[:, d],
                    start=(d == 0), stop=(d == GC - 1),
                )
            g_sb = small.tile([P, C], BF16, tag="g_sb")
            nc.scalar.copy(g_sb, psum_g)
            return g_sb

        def gate_chunk(b, i, g_sb):
            """512KB gate multiply + store for l-chunk i of sample b."""
            h, n = i // 2, i % 2
            g_bc = bass.AP(
                tensor=g_sb.tensor,
                offset=g_sb.offset,
                ap=[list(g_sb.ap[0]), [0, JC], list(g_sb.ap[1])],
            )
            o_t = xout.tile([P, JC * C], BF16, tag="o", name=f"o_b{b}_c{i}")
            nc.vector.tensor_tensor(
                out=o_t.rearrange("p (j c) -> p j c", j=JC),
                in0=xs[b][h][:, n].rearrange("p (j c) -> p j c", j=JC),
                in1=g_bc,
                op=mybir.AluOpType.mult,
            )
            nc.sync.dma_start(
                out=out_dsts[b][2 * h + n].rearrange("p x -> p x"),
                in_=o_t,
            )

        # ---- software-pipelined emission over samples ----
        pq = qkv_bias(0)
        qkv_half(0, 0, pq)
        qkv_half(0, 1, pq)
        cur_sb = qkv_copy(0, pq)
        for b in range(B_LOC):
            nxt_pq = None
            if b + 1 < B_LOC:
                nxt_pq = qkv_bias(b + 1)
                qkv_half(b + 1, 0, nxt_pq)
            g_sb = tail_stage(b, cur_sb)
            if b + 1 < B_LOC:
                qkv_half(b + 1, 1, nxt_pq)
                cur_sb = qkv_copy(b + 1, nxt_pq)
            for i in range(2 * (NCH // 2)):
                gate_chunk(b, i, g_sb)


def build():
    nc = bacc.Bacc(
        "TRN2", target_bir_lowering=False, debug=False, num_devices=N_CORES
    )
    x_d = nc.dram_tensor("x", [B_LOC, L, C], BF16, kind="ExternalInput")
    wT_d = nc.dram_tensor("wT", [L, TH], BF16, kind="ExternalInput")
    bias_d = nc.dram_tensor("bias", [1, TH], BF16, kind="ExternalInput")
    id_d = nc.dram_tensor("ident", [P, P], BF16, kind="ExternalInput")
    ones1_d = nc.dram_tensor("ones1", [1, P], BF16, kind="ExternalInput")
    onesg_d = nc.dram_tensor("onesg", [P, P], BF16, kind="ExternalInput")
    out_d = nc.dram_tensor("out", [B_LOC, L, C], BF16, kind="ExternalOutput")
    with tile.TileContext(nc) as tc:
        _emit(tc, x_d, wT_d, bias_d, id_d, ones1_d, onesg_d, out_d)
    nc.compile()
    return nc


_nc_cache = None


def _get_nc():
    global _nc_cache
    if _nc_cache is None:
        _nc_cache = build()
    return _nc_cache


def make_in_maps(x, Wq, bq, Wkv, bkv):
    x_bf = np.asarray(x, dtype=np.float32).astype(BF)
    wT = np.ascontiguousarray(
        np.concatenate(
            [np.asarray(Wq, np.float32) * SCALE, np.asarray(Wkv, np.float32)],
            axis=0,
        ).T.astype(BF)
    )
    bias = np.concatenate(
        [np.asarray(bq, np.float32) * SCALE, np.asarray(bkv, np.float32)]
    )[None].astype(BF)
    ident = np.eye(P, dtype=BF)
    ones1 = np.ones((1, P), dtype=BF)
    onesg = np.full((P, P), 1.0 / H, dtype=BF)
    return [
        {
            "x": np.ascontiguousarray(x_bf[i * B_LOC : (i + 1) * B_LOC]),
            "wT": wT,
            "bias": bias,
            "ident": ident,
            "ones1": ones1,
            "onesg": onesg,
        }
        for i in range(N_CORES)
    ]


def run(inputs, **spmd_kwargs):
    """Run on hardware; returns (full_output, BassKernelResults)."""
    nc = _get_nc()
    in_maps = make_in_maps(**inputs)
    res = run_bass_kernel_spmd(nc, in_maps, list(range(N_CORES)), **spmd_kwargs)
    out = np.concatenate([r["out"] for r in res.results], axis=0)
    return np.asarray(out).astype(np.float32), res


def kernel(**inputs) -> np.ndarray:
    out, _ = run(inputs)
    return out


# revision 7
# speedup vs baseline: 1.0826x; 1.0688x over previous
"""Trainium2 Bass kernel for nn_ChannelSelfAttention.

Reference computation (per batch sample b):
    xt   = x[b].T                          # [C, L]
    q    = xt @ Wq.T + bq                  # [C, H]
    kv   = xt @ Wkv.T + bkv                # [C, 2H] -> k, v
    attn = (q * H**-0.5) @ k.T             # [C, C]  (no softmax)
    y    = attn @ v                        # [C, H]
    g    = mean(y, axis=-1)                # [C]
    out[b] = x[b] * g[None, :]             # [L, C]

No softmax -> everything after qkv is LINEAR, so the [C,C] attention
matrix is never materialized:

    g[c] = sum_h q''[c,h] * kvb[h]
    kvb[h] = sum_d k[d,h] * vbar[d],  vbar[d] = sum_h' v[d,h']

with scale/H folded into Wq/bq on the host (q'' = q * scale / H).

Sharding: data-parallel over B across 8 cores (4 samples per core);
weights replicated.  All HBM I/O is bf16 (host casts): 17.5 MiB per
core, and the DMA stream runs at the per-NC HBM wall, so the schedule
is built to keep the sync ring 100% fed and the tail short:

  - Constants ride the GPSIMD (SWDGE) queue; bulk x/wT/out on sync
    with 8KB/6KB descriptors (j=16 rows per partition per chunk).
  - Loads issue before any store so store-gate semaphore waits on the
    SP sequencer can't delay a load.
  - PE warm-up is one continuous ~5us run of wide junk matmuls so the
    clock is at full p-state when the first real matmul issues.
  - PE emission is software-pipelined: qkv(b+1) half-0 between qkv(b)
    drain and sample b's tail matmuls, half-1 after.
  - vbar reduces read qkv directly from PSUM (no wait on the ACT copy)
    and are emitted ahead of the previous sample's gate multiplies so
    Tile's readiness scheduler never parks them behind 4.5us of DVE.
  - Per-sample tail chain: [ACT qkv copy || DVE vbar] -> PE (kvb via a
    stride-0 broadcast rhs of vbar, + q transposes, one shared PSUM
    tile) -> one ACT copy -> PE g matmul -> ACT g copy -> DVE gate
    multiplies (2 per 1MB half) -> 1MB stores.
"""

import numpy as np
import ml_dtypes

import concourse.bass as bass
import concourse.mybir as mybir
import concourse.tile as tile
from concourse import bacc
from concourse.bass_utils import run_bass_kernel_spmd

B, L, C, H = 32, 4096, 256, 64
N_CORES = 8
B_LOC = B // N_CORES          # samples per core
P = 128                       # SBUF partitions
JC = 16                       # L-rows per partition per chunk (8KB bf16 descs)
NCH = L // (P * JC)           # l-chunks per sample (2)
GC = C // P                   # c-groups (2)
TH = 3 * H                    # 192 = q|k|v
BF16 = mybir.dt.bfloat16
F32 = mybir.dt.float32
SCALE = float(H) ** -0.5
BF = ml_dtypes.bfloat16
N_WARM = 12                   # PE warm-up junk matmuls (512 cols each)
WCOL = 512


def _emit(tc: "tile.TileContext", x_d, wT_d, bias_d, id_d, ones1_d,
          out_d) -> None:
    nc = tc.nc
    with (
        tc.tile_pool(name="singles", bufs=1) as singles,
        tc.tile_pool(name="xin", bufs=B_LOC) as xin,
        tc.tile_pool(name="xout", bufs=7) as xout,
        tc.tile_pool(name="small", bufs=2) as small,
        tc.tile_pool(name="psW", bufs=1, space="PSUM") as psW,
        tc.tile_pool(name="psA", bufs=2, space="PSUM") as psA,
        tc.tile_pool(name="psA2", bufs=2, space="PSUM") as psA2,
        tc.tile_pool(name="psM", bufs=2, space="PSUM") as psM,
        tc.tile_pool(name="psG", bufs=1, space="PSUM") as psG,
    ):
        # ---- constants on the GPSIMD (SWDGE) queue: tiny, lands early,
        # steals nothing from the sync ring or the ACT/SP sequencers ----
        bias_sb = singles.tile([1, TH], BF16)            # (bq*scale/H)|bkv
        nc.gpsimd.dma_start(out=bias_sb, in_=bias_d[:])
        ones1 = singles.tile([1, P], BF16)               # ones row
        nc.gpsimd.dma_start(out=ones1, in_=ones1_d[:])
        ident = singles.tile([P, P], BF16)
        nc.gpsimd.dma_start(out=ident, in_=id_d[:])

        # ---- PE warm-up: one continuous run of wide junk matmuls so the
        # HAM clock-gate reaches full p-state before real work ----
        scratch = singles.tile([P, P + WCOL], BF16)
        nc.vector.memset(scratch, 0.0)
        psj = psW.tile([P, WCOL], F32, tag="warm", name="psj")
        for _ in range(N_WARM):
            nc.tensor.matmul(psj, lhsT=scratch[:, 0:P],
                             rhs=scratch[:, P : P + WCOL])

        # ---- bulk loads on the sync ring, in consumption order ----
        wT_src = wT_d[:].rearrange("(n p j) h -> p n (j h)", p=P, j=JC)
        x_srcs = [x_d[b].rearrange("(n p j) c -> n p (j c)", p=P, j=JC)
                  for b in range(B_LOC)]
        out_dsts = [out_d[b].rearrange("(n p j) c -> n p (j c)", p=P, j=JC)
                    for b in range(B_LOC)]
        wt = singles.tile([P, NCH, JC * TH], BF16)       # 1.5 MiB, 6KB descs
        nc.sync.dma_start(out=wt, in_=wT_src)
        xs = [[None] * NCH for _ in range(B_LOC)]
        for b in range(B_LOC):
            for h in range(NCH):
                xs[b][h] = xin.tile([P, JC * C], BF16,
                                    tag=f"x{h}", name=f"x_b{b}_h{h}")
                nc.sync.dma_start(
                    out=xs[b][h],
                    in_=x_srcs[b][h : h + 1].rearrange("n p x -> p (n x)"),
                )

        def qkv_bias(b):
            """Open sample b's PSUM accumulation groups with the bias."""
            pq = [psA.tile([P, TH], F32, tag="qkv0", name=f"pq0_b{b}"),
                  psA2.tile([P, TH], F32, tag="qkv1", name=f"pq1_b{b}")]
            for g in range(GC):
                nc.tensor.matmul(
                    pq[g], lhsT=ones1, rhs=bias_sb, start=True, stop=False,
                )
            return pq

        def qkv_half(b, h, pq):
            """Chunk matmuls for l-chunk h of sample b (x-stationary)."""
            for j in range(JC):
                for g in range(GC):
                    nc.tensor.matmul(
                        pq[g],
                        lhsT=xs[b][h][:, j * C + g * P : j * C + (g + 1) * P],
                        rhs=wt[:, h, j * TH : (j + 1) * TH],
                        start=False,
                        stop=(h == NCH - 1 and j == JC - 1),
                    )

        def qkv_copy(b, pq):
            qkv_sb = small.tile([P, GC, TH], BF16, tag="qkv_sb")
            for g in range(GC):
                nc.scalar.copy(qkv_sb[:, g], pq[g])
            return qkv_sb

        def vbar_stage(b, pq):
            """vbar[d] = sum_h v[d,h], read straight from the qkv PSUM."""
            vbar_sb = small.tile([P, GC, 1], BF16, tag="vbar")
            with nc.allow_low_precision(reason="bf16 vbar feeds bf16 matmul"):
                for g in range(GC):
                    nc.vector.tensor_reduce(
                        out=vbar_sb[:, g], in_=pq[g][:, 2 * H : TH],
                        axis=mybir.AxisListType.X, op=mybir.AluOpType.add,
                    )
            return vbar_sb

        def chain_stage(b, qkv_sb, vbar_sb):
            """kvb + q^T -> one PSUM tile -> one copy -> g matmul."""
            ps_m = psM.tile([H, P + C], F32, tag="m")
            # kvb[h] (broadcast along 128 free cols): lhsT = k-section,
            # rhs = vbar as a stride-0 broadcast row
            for g in range(GC):
                vb_bc = bass.AP(
                    tensor=vbar_sb.tensor,
                    offset=vbar_sb.offset + g * vbar_sb.ap[1][0],
                    ap=[list(vbar_sb.ap[0]), [0, P]],
                )
                nc.tensor.matmul(
                    ps_m[:, 0:P], lhsT=qkv_sb[:, g, H : 2 * H], rhs=vb_bc,
                    start=(g == 0), stop=(g == GC - 1),
                )
            # q^T [64, 256] via PE matmul against the identity
            for g in range(GC):
                nc.tensor.matmul(
                    ps_m[:, P + g * P : P + (g + 1) * P],
                    lhsT=qkv_sb[:, g, 0:H], rhs=ident,
                )
            m_sb = small.tile([H, P + C], BF16, tag="m_sb")
            nc.scalar.copy(m_sb, ps_m)
            # g[c] = sum_h kvb[h] qT[h, c], landing on all 128 partitions
            ps_g = psG.tile([P, C], F32, tag="g")
            nc.tensor.matmul(ps_g, lhsT=m_sb[:, 0:P], rhs=m_sb[:, P : P + C])
            g_sb = small.tile([P, C], BF16, tag="g_sb")
            nc.scalar.copy(g_sb, ps_g)
            return g_sb

        def gate_store(b, h, g_sb):
            """Gate multiply (2 DVE ops) + one 1MB store for half h."""
            g_bc = bass.AP(
                tensor=g_sb.tensor,
                offset=g_sb.offset,
                ap=[list(g_sb.ap[0]), [0, JC // 2], list(g_sb.ap[1])],
            )
            o_t = xout.tile([P, JC * C], BF16, tag="o", name=f"o_b{b}_h{h}")
            half = JC // 2 * C
            for jh in range(2):
                nc.vector.tensor_tensor(
                    out=o_t[:, jh * half : (jh + 1) * half]
                    .rearrange("p (j c) -> p j c", c=C),
                    in0=xs[b][h][:, jh * half : (jh + 1) * half]
                    .rearrange("p (j c) -> p j c", c=C),
                    in1=g_bc,
                    op=mybir.AluOpType.mult,
                )
            nc.sync.dma_start(
                out=out_dsts[b][h].rearrange("p x -> p x"),
                in_=o_t,
            )

        # ---- software-pipelined emission over samples ----
        pq = qkv_bias(0)
        qkv_half(0, 0, pq)
        qkv_half(0, 1, pq)
        cur_sb = qkv_copy(0, pq)
        cur_vb = vbar_stage(0, pq)
        for b in range(B_LOC):
            nxt_pq = None
            if b + 1 < B_LOC:
                nxt_pq = qkv_bias(b + 1)
                qkv_half(b + 1, 0, nxt_pq)
            g_sb = chain_stage(b, cur_sb, cur_vb)
            if b + 1 < B_LOC:
                qkv_half(b + 1, 1, nxt_pq)
                cur_sb = qkv_copy(b + 1, nxt_pq)
                cur_vb = vbar_stage(b + 1, nxt_pq)
            for h in range(NCH):
                gate_store(b, h, g_sb)


def build():
    nc = bacc.Bacc(
        "TRN2", target_bir_lowering=False, debug=False, num_devices=N_CORES
    )
    x_d = nc.dram_tensor("x", [B_LOC, L, C], BF16, kind="ExternalInput")
    wT_d = nc.dram_tensor("wT", [L, TH], BF16, kind="ExternalInput")
    bias_d = nc.dram_tensor("bias", [1, TH], BF16, kind="ExternalInput")
    id_d = nc.dram_tensor("ident", [P, P], BF16, kind="ExternalInput")
    ones1_d = nc.dram_tensor("ones1", [1, P], BF16, kind="ExternalInput")
    out_d = nc.dram_tensor("out", [B_LOC, L, C], BF16, kind="ExternalOutput")
    with tile.TileContext(nc) as tc:
        _emit(tc, x_d, wT_d, bias_d, id_d, ones1_d, out_d)
    nc.compile()
    return nc


_nc_cache = None


def _get_nc():
    global _nc_cache
    if _nc_cache is None:
        _nc_cache = build()
    return _nc_cache


def make_in_maps(x, Wq, bq, Wkv, bkv):
    x_bf = np.asarray(x, dtype=np.float32).astype(BF)
    qs = SCALE / H                      # fold attn scale AND mean-over-H into q
    wT = np.ascontiguousarray(
        np.concatenate(
            [np.asarray(Wq, np.float32) * qs, np.asarray(Wkv, np.float32)],
            axis=0,
        ).T.astype(BF)
    )
    bias = np.concatenate(
        [np.asarray(bq, np.float32) * qs, np.asarray(bkv, np.float32)]
    )[None].astype(BF)
    ident = np.eye(P, dtype=BF)
    ones1 = np.ones((1, P), dtype=BF)
    return [
        {
            "x": np.ascontiguousarray(x_bf[i * B_LOC : (i + 1) * B_LOC]),
            "wT": wT,
            "bias": bias,
            "ident": ident,
            "ones1": ones1,
        }
        for i in range(N_CORES)
    ]


def run(inputs, **spmd_kwargs):
    """Run on hardware; returns (full_output, BassKernelResults)."""
    nc = _get_nc()
    in_maps = make_in_maps(**inputs)
    res = run_bass_kernel_spmd(nc, in_maps, list(range(N_CORES)), **spmd_kwargs)
    out = np.concatenate([r["out"] for r in res.results], axis=0)
    return np.asarray(out).astype(np.float32), res


def kernel(**inputs) -> np.ndarray:
    out, _ = run(inputs)
    return out
